# revision 20
# baseline (speedup 1.0000x reference)
"""Local multi-headed attention (window +/-2) + residual + LayerNorm, Trainium2 Bass kernel.

Sharding: data-parallel over batch. B=8 batch elements -> one per NeuronCore (8 cores).
Each core computes the full sequence for its batch element; no collectives.

v2 layout strategy (vs v1 baseline):
  - Projections as v1: xT via PE transpose, W.T @ xT -> qT/kT/vT [d,s], bf16,
    PSUM accumulate, ScalarE PSUM->SBUF copy fused with bias.
  - kcB/vcB = kcA/vcA shifted by one element (DVE 4x copies) so every tap slice is
    4B-aligned and tensor_tensor ops keep the 2x bf16 perf mode.
  - Scores: prod_w = qT * shift_w(kT) merged over all 6 d-tiles (FD=1536, 2x mode),
    then per-dt blockones matmuls (reduce 64 head dims + broadcast) and one exp
    (scale=1/8) per dt writing into ex_all [128, 6, 5, 256].
  - Softmax denominator: 4 DVE bf16 adds over tap slices of ex_all (dt-merged),
    reciprocal_approx_fast, bf16 rinv; AV = sum_w ex_w * v_w (dt-merged mults/adds),
    final att = asum * rinv.
  - O-projection as v1 (att blocks stationary vs Wo, bias via ones-row matmul).
  - Residual add on GPSIMD (PSUM + x), LN stats via bn_stats/bn_aggr on DVE,
    rstd = Exp(-0.5*Ln(var+eps)) on ScalarE so the whole kernel uses ONE activation
    table set (natural_log_exp_and_others) -- no ACT_TABLE_LOAD thrash.
  - gamma/beta application on GPSIMD from partition-broadcast copies.
"""
import os
import sys
import numpy as np

B, S, D = 8, 4096, 768
HEADS = 12
DH = 64
W = 5          # window taps, offsets -2..2
CHUNK = 256    # sequence chunk per inner iteration
NCH = S // CHUNK
DT = D // 128  # 6 partition tiles of d
KW = CHUNK + 6  # kc/vc tile width: 2 halo left, 2 halo right, 2 align pad
EPS = 1e-5
N_CORES = 8

_cache = {}


def _build():
    import concourse.bass as bass
    import concourse.tile as tile
    from concourse import bacc, mybir
    from concourse.masks import make_identity

    f32 = mybir.dt.float32
    bf16 = mybir.dt.bfloat16
    AF = mybir.ActivationFunctionType
    ALU = mybir.AluOpType

    nc = bacc.Bacc("TRN2", target_bir_lowering=False, debug=False,
                   num_devices=N_CORES)

    x_ap = nc.dram_tensor("x", [S, D], f32, kind="ExternalInput").ap()
    wq_ap = nc.dram_tensor("Wq", [D, D], f32, kind="ExternalInput").ap()
    bq_ap = nc.dram_tensor("bq", [D], f32, kind="ExternalInput").ap()
    wk_ap = nc.dram_tensor("Wk", [D, D], f32, kind="ExternalInput").ap()
    bk_ap = nc.dram_tensor("bk", [D], f32, kind="ExternalInput").ap()
    wv_ap = nc.dram_tensor("Wv", [D, D], f32, kind="ExternalInput").ap()
    bv_ap = nc.dram_tensor("bv", [D], f32, kind="ExternalInput").ap()
    wo_ap = nc.dram_tensor("Wo", [D, D], f32, kind="ExternalInput").ap()
    bo_ap = nc.dram_tensor("bo", [D], f32, kind="ExternalInput").ap()
    gamma_ap = nc.dram_tensor("gamma", [D], f32, kind="ExternalInput").ap()
    beta_ap = nc.dram_tensor("beta", [D], f32, kind="ExternalInput").ap()
    out_ap = nc.dram_tensor("out", [S, D], f32, kind="ExternalOutput").ap()

    with tile.TileContext(nc) as tc:
        # ---------------- persistent tiles ----------------
        with tc.tile_pool(name="persist", bufs=1) as pp:
            wq_sb = pp.tile([128, DT, D], bf16, tag="wq")
            wk_sb = pp.tile([128, DT, D], bf16, tag="wk")
            wv_sb = pp.tile([128, DT, D], bf16, tag="wv")
            wo_sb = pp.tile([128, DT, D], bf16, tag="wo")
            bqT = pp.tile([128, DT], f32, tag="bqT")
            bkT = pp.tile([128, DT], f32, tag="bkT")
            bvT = pp.tile([128, DT], f32, tag="bvT")
            bo_sb = pp.tile([1, D], f32, tag="bo")
            g_sb = pp.tile([1, D], f32, tag="g")
            be_sb = pp.tile([1, D], f32, tag="be")
            ones_row = pp.tile([1, 128], f32, tag="ones")
            ones_bf = pp.tile([1, 128], bf16, tag="onesbf")
            bo_bf = pp.tile([1, D], bf16, tag="bobf")
            blockones = pp.tile([128, 128], bf16, tag="bones")
            ident = pp.tile([128, 128], f32, tag="ident")
            gb_bc = pp.tile([128, D], f32, tag="gbbc")
            be_bc = pp.tile([128, D], f32, tag="bebc")
            eps_sb = pp.tile([128, 1], f32, tag="eps")
            nc.vector.memset(eps_sb[:], EPS)
            c15_sb = pp.tile([128, 1], f32, tag="c15")
            nc.vector.memset(c15_sb[:], 1.5)

            with tc.tile_pool(name="wstagep", bufs=2) as wsp:
                for w_ap, sb in ((wq_ap, wq_sb), (wk_ap, wk_sb), (wv_ap, wv_sb),
                                 (wo_ap, wo_sb)):
                    st = wsp.tile([128, DT, D], f32, tag="wstage")
                    nc.sync.dma_start(st[:],
                                      w_ap.rearrange("(kt p) n -> p kt n", p=128))
                    nc.vector.tensor_copy(sb[:], st[:])
            nc.sync.dma_start(bqT[:], bq_ap.rearrange("(t p) -> p t", p=128))
            nc.sync.dma_start(bkT[:], bk_ap.rearrange("(t p) -> p t", p=128))
            nc.sync.dma_start(bvT[:], bv_ap.rearrange("(t p) -> p t", p=128))
            nc.sync.dma_start(bo_sb[:], bo_ap[:])
            nc.sync.dma_start(g_sb[:], gamma_ap[:])
            nc.sync.dma_start(be_sb[:], beta_ap[:])

            nc.vector.memset(ones_row[:], 1.0)
            nc.vector.memset(ones_bf[:], 1.0)
            nc.vector.tensor_copy(bo_bf[:], bo_sb[:])
            nc.vector.memset(blockones[:], 0.0)
            nc.vector.memset(blockones[0:64, 0:64], 1.0)
            nc.vector.memset(blockones[64:128, 64:128], 1.0)
            make_identity(nc, ident[:])

            # broadcast gamma/beta across partitions via K=1 matmul
            with tc.tile_pool(name="initps", bufs=1, space="PSUM") as initps:
                for src, dst in ((g_sb, gb_bc), (be_sb, be_bc)):
                    t = initps.tile([128, D], f32, tag="gbps")
                    nc.tensor.matmul(t[:, 0:512], ones_row[:], src[:, 0:512])
                    nc.tensor.matmul(t[:, 512:D], ones_row[:], src[:, 512:D])
                    nc.vector.tensor_copy(dst[:], t[:])

            # ---------------- working pools ----------------
            with tc.tile_pool(name="ppsum", bufs=3, space="PSUM") as ppsum, \
                 tc.tile_pool(name="spsum", bufs=3, space="PSUM") as spsum, \
                 tc.tile_pool(name="opsum", bufs=1, space="PSUM") as opsum, \
                 tc.tile_pool(name="xpool", bufs=3) as xpool, \
                 tc.tile_pool(name="xtpool", bufs=2) as xtpool, \
                 tc.tile_pool(name="qpool", bufs=3) as qpool, \
                 tc.tile_pool(name="kvpool", bufs=3) as kvpool, \
                 tc.tile_pool(name="kvbpool", bufs=1) as kvbpool, \
                 tc.tile_pool(name="prodpool", bufs=5) as prodpool, \
                 tc.tile_pool(name="expool", bufs=2) as expool, \
                 tc.tile_pool(name="avpool", bufs=1) as avpool, \
                 tc.tile_pool(name="atpool", bufs=2) as atpool, \
                 tc.tile_pool(name="dnpool", bufs=1) as dnpool, \
                 tc.tile_pool(name="ypool", bufs=2) as ypool, \
                 tc.tile_pool(name="stpool", bufs=2) as stpool:

                kc_tiles = [None] * NCH
                vc_tiles = [None] * NCH

                def project(c):
                    """projections for chunk c -> qT (bf16) and kc/vc center cols."""
                    s0 = c * CHUNK
                    x_sb = xpool.tile([128, 2, D], f32, tag="x")
                    nc.sync.dma_start(
                        x_sb[:], x_ap[s0:s0 + CHUNK, :].rearrange(
                            "(st p) d -> p st d", p=128))
                    # transpose to xT bf16 [128, DT, CHUNK]
                    xT = xtpool.tile([128, DT, CHUNK], bf16, tag="xT")
                    for dt in range(DT):
                        tp = ppsum.tile([128, CHUNK], f32, tag="proj")
                        for st in range(2):
                            nc.tensor.transpose(
                                tp[:, st * 128:(st + 1) * 128],
                                x_sb[:, st, dt * 128:(dt + 1) * 128], ident[:])
                        nc.scalar.copy(xT[:, dt, :], tp[:])

                    qT = qpool.tile([128, DT, CHUNK], bf16, tag="qT")
                    kc = kvpool.tile([128, DT, KW], bf16, tag="kc")
                    vc = kvpool.tile([128, DT, KW], bf16, tag="vc")
                    kc_tiles[c] = kc
                    vc_tiles[c] = vc
                    for (wsb, bT, dst, off) in ((wq_sb, bqT, qT, None),
                                                (wk_sb, bkT, kc, 2),
                                                (wv_sb, bvT, vc, 2)):
                        for dt in range(DT):
                            ps = ppsum.tile([128, CHUNK], f32, tag="proj")
                            for kt in range(DT):
                                nc.tensor.matmul(
                                    ps[:],
                                    wsb[:, kt, dt * 128:(dt + 1) * 128],
                                    xT[:, kt, :],
                                    start=(kt == 0), stop=(kt == DT - 1))
                            dslice = dst[:, dt, :] if off is None \
                                else dst[:, dt, 2:2 + CHUNK]
                            nc.scalar.activation(dslice, ps[:], AF.Identity,
                                                 bias=bT[:, dt:dt + 1])
                    # halo fills
                    if c > 0:
                        for big_prev, big_cur in ((kc_tiles[c - 1], kc),
                                                  (vc_tiles[c - 1], vc)):
                            nc.vector.tensor_copy(big_cur[:, :, 0:2],
                                                  big_prev[:, :, CHUNK:CHUNK + 2])
                            nc.vector.tensor_copy(big_prev[:, :, CHUNK + 2:CHUNK + 4],
                                                  big_cur[:, :, 2:4])
                    if c == 0:
                        for big, bT in ((kc, bkT), (vc, bvT)):
                            for dt in range(DT):
                                nc.vector.memset(big[:, dt, 0:2], 0.0)
                                nc.scalar.activation(big[:, dt, 0:2],
                                                     big[:, dt, 0:2],
                                                     AF.Identity,
                                                     bias=bT[:, dt:dt + 1])
                    if c == NCH - 1:
                        for big, bT in ((kc, bkT), (vc, bvT)):
                            for dt in range(DT):
                                nc.vector.memset(big[:, dt, CHUNK + 2:CHUNK + 4], 0.0)
                                nc.scalar.activation(big[:, dt, CHUNK + 2:CHUNK + 4],
                                                     big[:, dt, CHUNK + 2:CHUNK + 4],
                                                     AF.Identity,
                                                     bias=bT[:, dt:dt + 1])
                    # zero the 2-col align pad so the shifted copies read defined data
                    nc.vector.memset(kc[:, :, CHUNK + 4:KW], 0.0)
                    nc.vector.memset(vc[:, :, CHUNK + 4:KW], 0.0)
                    return x_sb, qT

                def attention(c, x_sb, qT):
                    """scores/softmax/AV/O-proj/LN for chunk c (projections done)."""
                    s0 = c * CHUNK
                    kc, vc = kc_tiles[c], vc_tiles[c]
                    # odd-tap alignment shadows: kcB[j] = kc[j+1] (4x-mode copies)
                    kcB = kvbpool.tile([128, DT, KW - 2], bf16, tag="kcB")
                    vcB = kvbpool.tile([128, DT, KW - 2], bf16, tag="vcB")
                    nc.vector.tensor_copy(kcB[:], kc[:, :, 1:KW - 1])
                    nc.vector.tensor_copy(vcB[:], vc[:, :, 1:KW - 1])

                    def tap(big, bigB, w):
                        """[128, DT, CHUNK] slice for tap w, always 4B-aligned."""
                        if w % 2 == 0:
                            return big[:, :, w:w + CHUNK]
                        return bigB[:, :, w - 1:w - 1 + CHUNK]

                    # prods: dt-merged elementwise q*k per tap (bf16 2x mode)
                    prods = []
                    for w in range(W):
                        pr = prodpool.tile([128, DT, CHUNK], bf16, tag="prod")
                        nc.vector.tensor_tensor(pr[:], qT[:], tap(kc, kcB, w),
                                                ALU.mult)
                        prods.append(pr)
                    # scores + head-reduce + broadcast; 2-tap 1-bank PSUM
                    # tiles so score matmuls pipeline with the exp activations
                    ex_all = expool.tile([128, DT, W, CHUNK], bf16, tag="ex")
                    for dt in range(DT):
                        for w0 in (0, 2, 4):
                            nw = 2 if w0 < 4 else 1
                            sc = spsum.tile([128, 2, CHUNK], f32, tag="scores")
                            for w in range(w0, w0 + nw):
                                nc.tensor.matmul(sc[:, w - w0, :], blockones[:],
                                                 prods[w][:, dt, :])
                            nc.scalar.activation(
                                ex_all[:, dt, w0:w0 + nw, :],
                                sc[:, 0:nw, :], AF.Exp, scale=0.125)
                    # softmax denominator (dt-merged bf16 adds) + reciprocal
                    dn = dnpool.tile([128, DT, CHUNK], bf16, tag="dn")
                    dnf = dnpool.tile([128, DT, CHUNK], f32, tag="dnf")
                    rinv = dnpool.tile([128, DT, CHUNK], f32, tag="rinv")
                    rinvb = dnpool.tile([128, DT, CHUNK], bf16, tag="rinvb")
                    nc.vector.tensor_tensor(dn[:], ex_all[:, :, 0, :],
                                            ex_all[:, :, 1, :], ALU.add)
                    nc.vector.tensor_tensor(dn[:], dn[:], ex_all[:, :, 2, :],
                                            ALU.add)
                    nc.vector.tensor_tensor(dn[:], dn[:], ex_all[:, :, 3, :],
                                            ALU.add)
                    nc.vector.tensor_tensor(dnf[:], dn[:], ex_all[:, :, 4, :],
                                            ALU.add)
                    nc.vector.reciprocal_approx_fast(rinv[:], dnf[:])
                    nc.vector.tensor_copy(rinvb[:], rinv[:])
                    # AV: avp_w = exp_w * v_tap_w (dt-merged), tree-sum, normalize
                    att = atpool.tile([128, DT, CHUNK], bf16, tag="att")
                    asum = avpool.tile([128, DT, CHUNK], bf16, tag="asum")
                    avp = avpool.tile([128, DT, CHUNK], bf16, tag="avp")
                    nc.vector.tensor_tensor(asum[:], ex_all[:, :, 0, :],
                                            tap(vc, vcB, 0), ALU.mult)
                    for w in range(1, W):
                        nc.vector.tensor_tensor(avp[:], ex_all[:, :, w, :],
                                                tap(vc, vcB, w), ALU.mult)
                        nc.vector.tensor_tensor(asum[:], asum[:], avp[:], ALU.add)
                    nc.vector.tensor_tensor(att[:], asum[:], rinvb[:], ALU.mult)

                    # O-projection + bias + residual + LayerNorm per s-tile
                    for st in range(2):
                        op = opsum.tile([128, D], f32, tag="o")
                        for dt in range(DT):
                            a_blk = att[:, dt, st * 128:(st + 1) * 128]
                            nc.tensor.matmul(op[:, 0:512], a_blk,
                                             wo_sb[:, dt, 0:512],
                                             start=(dt == 0), stop=False)
                            nc.tensor.matmul(op[:, 512:D], a_blk,
                                             wo_sb[:, dt, 512:D],
                                             start=(dt == 0), stop=False)
                        nc.tensor.matmul(op[:, 0:512], ones_bf[:],
                                         bo_bf[:, 0:512], start=False, stop=True)
                        nc.tensor.matmul(op[:, 512:D], ones_bf[:],
                                         bo_bf[:, 512:D], start=False, stop=True)
                        # residual (DVE: GPSIMD has no PSUM port)
                        ypre = ypool.tile([128, D], f32, tag="ypre")
                        nc.vector.tensor_tensor(ypre[:], op[:], x_sb[:, st, :],
                                                ALU.add)
                        # LayerNorm stats via bn_stats/bn_aggr (DVE one pass)
                        bns = stpool.tile([128, 2, 6], f32, tag="bns")
                        agg = stpool.tile([128, 2], f32, tag="agg")
                        stats = stpool.tile([128, 6], f32, tag="stats")
                        sti = stpool.tile([128, 2], mybir.dt.int32, tag="sti")
                        nc.vector.bn_stats(bns[:, 0, :], ypre[:, 0:384])
                        nc.vector.bn_stats(bns[:, 1, :], ypre[:, 384:D])
                        nc.vector.bn_aggr(agg[:], bns[:])
                        # rstd = rsqrt(var+eps) via bit-trick seed + 2 Newton
                        # iterations, all on GPSIMD (keeps ScalarE on one
                        # activation table set -- no ACT_TABLE_LOAD thrash)
                        xe = stats[:, 0:1]
                        y = stats[:, 1:2]
                        t = stats[:, 2:3]
                        h = stats[:, 3:4]
                        negmu = stats[:, 4:5]
                        nmr = stats[:, 5:6]
                        nc.gpsimd.tensor_tensor(xe, agg[:, 1:2], eps_sb[:],
                                                ALU.add)
                        nc.vector.tensor_scalar(sti[:, 0:1], xe.bitcast(
                            mybir.dt.int32), 1, None, ALU.arith_shift_right)
                        nc.vector.tensor_scalar(sti[:, 1:2], sti[:, 0:1], -1,
                                                0x5f3759df, ALU.mult, ALU.add)
                        nc.vector.tensor_scalar_mul(h, xe, -0.5)
                        nc.vector.tensor_scalar_mul(negmu, agg[:, 0:1], -1.0)
                        yseed = sti[:, 1:2].bitcast(f32)
                        # one NR iter: y = y0*(1.5 + h*y0*y0), h = -0.5*(var+eps)
                        # (seed err ~3.4% -> ~1.7e-3 after NR; fine vs 2e-2 gate)
                        nc.gpsimd.tensor_tensor(t, yseed, yseed, ALU.mult)
                        nc.gpsimd.tensor_tensor(t, t, h, ALU.mult)
                        nc.gpsimd.tensor_tensor(t, t, c15_sb[:], ALU.add)
                        nc.gpsimd.tensor_tensor(y, yseed, t, ALU.mult)
                        # negmurstd = -mean * rstd
                        nc.gpsimd.tensor_tensor(nmr, negmu, y, ALU.mult)
                        y1 = ypool.tile([128, D], f32, tag="y1")
                        nc.scalar.activation(y1[:], ypre[:], AF.Identity,
                                             bias=nmr,
                                             scale=y)
                        y2 = ypool.tile([128, D], f32, tag="y2")
                        nc.gpsimd.tensor_tensor(y2[:], y1[:], gb_bc[:], ALU.mult)
                        nc.gpsimd.tensor_tensor(y2[:], y2[:], be_bc[:], ALU.add)
                        nc.sync.dma_start(
                            out_ap[s0 + st * 128: s0 + (st + 1) * 128, :], y2[:])

                # run projections TWO chunks ahead of attention: attention(c)
                # then never waits on fresh projections (kcB needs the right
                # halo written by project(c+1))
                pend = []
                for c in range(NCH):
                    pend.append(project(c))
                    if c >= 2:
                        attention(c - 2, *pend[c - 2])
                attention(NCH - 2, *pend[NCH - 2])
                attention(NCH - 1, *pend[NCH - 1])

    nc.compile()
    return nc


def kernel(**inputs):
    if "nc" not in _cache:
        _cache["nc"] = _build()
    nc = _cache["nc"]
    from concourse.bass_utils import run_bass_kernel_spmd

    names = ["Wq", "bq", "Wk", "bk", "Wv", "bv", "Wo", "bo", "gamma", "beta"]
    shared = {n: np.ascontiguousarray(np.asarray(inputs[n], dtype=np.float32))
              for n in names}
    x = np.asarray(inputs["x"], dtype=np.float32)
    in_maps = [dict(shared, x=np.ascontiguousarray(x[b])) for b in range(N_CORES)]
    res = run_bass_kernel_spmd(nc, in_maps, core_ids=list(range(N_CORES)))
    out = np.stack([res.results[i]["out"] for i in range(N_CORES)], axis=0)
    return out.astype(np.float32)


# revision 22
# speedup vs baseline: 1.1915x; 1.1915x over previous
"""Local multi-headed attention (window +/-2) + residual + LayerNorm, Trainium2 Bass kernel.

Sharding: data-parallel over batch. B=8 batch elements -> one per NeuronCore (8 cores).
Each core computes the full sequence for its batch element; no collectives.

v2 layout strategy (vs v1 baseline):
  - Projections as v1: xT via PE transpose, W.T @ xT -> qT/kT/vT [d,s], bf16,
    PSUM accumulate, ScalarE PSUM->SBUF copy fused with bias.
  - kcB/vcB = kcA/vcA shifted by one element (DVE 4x copies) so every tap slice is
    4B-aligned and tensor_tensor ops keep the 2x bf16 perf mode.
  - Scores: prod_w = qT * shift_w(kT) merged over all 6 d-tiles (FD=1536, 2x mode),
    then per-dt blockones matmuls (reduce 64 head dims + broadcast) and one exp
    (scale=1/8) per dt writing into ex_all [128, 6, 5, 256].
  - Softmax denominator: 4 DVE bf16 adds over tap slices of ex_all (dt-merged),
    reciprocal_approx_fast, bf16 rinv; AV = sum_w ex_w * v_w (dt-merged mults/adds),
    final att = asum * rinv.
  - O-projection as v1 (att blocks stationary vs Wo, bias via ones-row matmul).
  - Residual add on GPSIMD (PSUM + x), LN stats via bn_stats/bn_aggr on DVE,
    rstd = Exp(-0.5*Ln(var+eps)) on ScalarE so the whole kernel uses ONE activation
    table set (natural_log_exp_and_others) -- no ACT_TABLE_LOAD thrash.
  - gamma/beta application on GPSIMD from partition-broadcast copies.
"""
import os
import sys
import numpy as np

B, S, D = 8, 4096, 768
HEADS = 12
DH = 64
W = 5          # window taps, offsets -2..2
CHUNK = 256    # sequence chunk per inner iteration
NCH = S // CHUNK
DT = D // 128  # 6 partition tiles of d
KW = CHUNK + 6  # kc/vc tile width: 2 halo left, 2 halo right, 2 align pad
EPS = 1e-5
N_CORES = 8

_cache = {}


def _build():
    import concourse.bass as bass
    import concourse.tile as tile
    from concourse import bacc, mybir
    from concourse.masks import make_identity

    f32 = mybir.dt.float32
    bf16 = mybir.dt.bfloat16
    AF = mybir.ActivationFunctionType
    ALU = mybir.AluOpType

    nc = bacc.Bacc("TRN2", target_bir_lowering=False, debug=False,
                   num_devices=N_CORES)

    x_ap = nc.dram_tensor("x", [S, D], f32, kind="ExternalInput").ap()
    wq_ap = nc.dram_tensor("Wq", [D, D], f32, kind="ExternalInput").ap()
    bq_ap = nc.dram_tensor("bq", [D], f32, kind="ExternalInput").ap()
    wk_ap = nc.dram_tensor("Wk", [D, D], f32, kind="ExternalInput").ap()
    bk_ap = nc.dram_tensor("bk", [D], f32, kind="ExternalInput").ap()
    wv_ap = nc.dram_tensor("Wv", [D, D], f32, kind="ExternalInput").ap()
    bv_ap = nc.dram_tensor("bv", [D], f32, kind="ExternalInput").ap()
    wo_ap = nc.dram_tensor("Wo", [D, D], f32, kind="ExternalInput").ap()
    bo_ap = nc.dram_tensor("bo", [D], f32, kind="ExternalInput").ap()
    gamma_ap = nc.dram_tensor("gamma", [D], f32, kind="ExternalInput").ap()
    beta_ap = nc.dram_tensor("beta", [D], f32, kind="ExternalInput").ap()
    out_ap = nc.dram_tensor("out", [S, D], f32, kind="ExternalOutput").ap()

    with tile.TileContext(nc) as tc:
        # ---------------- persistent tiles ----------------
        with tc.tile_pool(name="persist", bufs=1) as pp:
            wq_sb = pp.tile([128, DT, D], bf16, tag="wq")
            wk_sb = pp.tile([128, DT, D], bf16, tag="wk")
            wv_sb = pp.tile([128, DT, D], bf16, tag="wv")
            wo_sb = pp.tile([128, DT, D], bf16, tag="wo")
            bqT = pp.tile([128, DT], f32, tag="bqT")
            bkT = pp.tile([128, DT], f32, tag="bkT")
            bvT = pp.tile([128, DT], f32, tag="bvT")
            bo_sb = pp.tile([1, D], f32, tag="bo")
            g_sb = pp.tile([1, D], f32, tag="g")
            be_sb = pp.tile([1, D], f32, tag="be")
            ones_row = pp.tile([1, 128], f32, tag="ones")
            ones_bf = pp.tile([1, 128], bf16, tag="onesbf")
            bo_bf = pp.tile([1, D], bf16, tag="bobf")
            blockones = pp.tile([128, 128], bf16, tag="bones")
            ident = pp.tile([128, 128], f32, tag="ident")
            gb_bc = pp.tile([128, D], f32, tag="gbbc")
            be_bc = pp.tile([128, D], f32, tag="bebc")
            eps_sb = pp.tile([128, 1], f32, tag="eps")
            nc.vector.memset(eps_sb[:], EPS)
            c15_sb = pp.tile([128, 1], f32, tag="c15")
            nc.vector.memset(c15_sb[:], 1.5)

            with tc.tile_pool(name="wstagep", bufs=2) as wsp:
                for w_ap, sb in ((wq_ap, wq_sb), (wk_ap, wk_sb), (wv_ap, wv_sb),
                                 (wo_ap, wo_sb)):
                    st = wsp.tile([128, DT, D], f32, tag="wstage")
                    nc.sync.dma_start(st[:],
                                      w_ap.rearrange("(kt p) n -> p kt n", p=128))
                    nc.vector.tensor_copy(sb[:], st[:])
            nc.sync.dma_start(bqT[:], bq_ap.rearrange("(t p) -> p t", p=128))
            nc.sync.dma_start(bkT[:], bk_ap.rearrange("(t p) -> p t", p=128))
            nc.sync.dma_start(bvT[:], bv_ap.rearrange("(t p) -> p t", p=128))
            nc.sync.dma_start(bo_sb[:], bo_ap[:])
            nc.sync.dma_start(g_sb[:], gamma_ap[:])
            nc.sync.dma_start(be_sb[:], beta_ap[:])

            nc.vector.memset(ones_row[:], 1.0)
            nc.vector.memset(ones_bf[:], 1.0)
            nc.vector.tensor_copy(bo_bf[:], bo_sb[:])
            nc.vector.memset(blockones[:], 0.0)
            nc.vector.memset(blockones[0:64, 0:64], 1.0)
            nc.vector.memset(blockones[64:128, 64:128], 1.0)
            make_identity(nc, ident[:])

            # broadcast gamma/beta across partitions via K=1 matmul
            with tc.tile_pool(name="initps", bufs=1, space="PSUM") as initps:
                for src, dst in ((g_sb, gb_bc), (be_sb, be_bc)):
                    t = initps.tile([128, D], f32, tag="gbps")
                    nc.tensor.matmul(t[:, 0:512], ones_row[:], src[:, 0:512])
                    nc.tensor.matmul(t[:, 512:D], ones_row[:], src[:, 512:D])
                    nc.vector.tensor_copy(dst[:], t[:])

            # ---------------- working pools ----------------
            with tc.tile_pool(name="ppsum", bufs=3, space="PSUM") as ppsum, \
                 tc.tile_pool(name="spsum", bufs=3, space="PSUM") as spsum, \
                 tc.tile_pool(name="opsum", bufs=1, space="PSUM") as opsum, \
                 tc.tile_pool(name="xpool", bufs=3) as xpool, \
                 tc.tile_pool(name="xtpool", bufs=2) as xtpool, \
                 tc.tile_pool(name="qpool", bufs=3) as qpool, \
                 tc.tile_pool(name="kvpool", bufs=3) as kvpool, \
                 tc.tile_pool(name="kvbpool", bufs=1) as kvbpool, \
                 tc.tile_pool(name="prodpool", bufs=5) as prodpool, \
                 tc.tile_pool(name="expool", bufs=2) as expool, \
                 tc.tile_pool(name="avpool", bufs=1) as avpool, \
                 tc.tile_pool(name="atpool", bufs=2) as atpool, \
                 tc.tile_pool(name="dnpool", bufs=1) as dnpool, \
                 tc.tile_pool(name="ypool", bufs=2) as ypool, \
                 tc.tile_pool(name="stpool", bufs=2) as stpool:

                kc_tiles = [None] * NCH
                vc_tiles = [None] * NCH

                def project(c):
                    """projections for chunk c -> qT (bf16) and kc/vc center cols."""
                    s0 = c * CHUNK
                    x_sb = xpool.tile([128, 2, D], f32, tag="x")
                    nc.sync.dma_start(
                        x_sb[:], x_ap[s0:s0 + CHUNK, :].rearrange(
                            "(st p) d -> p st d", p=128))
                    # transpose to xT bf16 [128, DT, CHUNK]
                    xT = xtpool.tile([128, DT, CHUNK], bf16, tag="xT")
                    for dt in range(DT):
                        tp = ppsum.tile([128, CHUNK], f32, tag="proj")
                        for st in range(2):
                            nc.tensor.transpose(
                                tp[:, st * 128:(st + 1) * 128],
                                x_sb[:, st, dt * 128:(dt + 1) * 128], ident[:])
                        nc.scalar.copy(xT[:, dt, :], tp[:])

                    qT = qpool.tile([128, DT, CHUNK], bf16, tag="qT")
                    kc = kvpool.tile([128, DT, KW], bf16, tag="kc")
                    vc = kvpool.tile([128, DT, KW], bf16, tag="vc")
                    kc_tiles[c] = kc
                    vc_tiles[c] = vc
                    for (wsb, bT, dst, off) in ((wq_sb, bqT, qT, None),
                                                (wk_sb, bkT, kc, 2),
                                                (wv_sb, bvT, vc, 2)):
                        for dt in range(DT):
                            ps = ppsum.tile([128, CHUNK], f32, tag="proj")
                            for kt in range(DT):
                                nc.tensor.matmul(
                                    ps[:],
                                    wsb[:, kt, dt * 128:(dt + 1) * 128],
                                    xT[:, kt, :],
                                    start=(kt == 0), stop=(kt == DT - 1))
                            dslice = dst[:, dt, :] if off is None \
                                else dst[:, dt, 2:2 + CHUNK]
                            nc.scalar.activation(dslice, ps[:], AF.Identity,
                                                 bias=bT[:, dt:dt + 1])
                    # halo fills
                    if c > 0:
                        for big_prev, big_cur in ((kc_tiles[c - 1], kc),
                                                  (vc_tiles[c - 1], vc)):
                            nc.vector.tensor_copy(big_cur[:, :, 0:2],
                                                  big_prev[:, :, CHUNK:CHUNK + 2])
                            nc.vector.tensor_copy(big_prev[:, :, CHUNK + 2:CHUNK + 4],
                                                  big_cur[:, :, 2:4])
                    if c == 0:
                        for big, bT in ((kc, bkT), (vc, bvT)):
                            for dt in range(DT):
                                nc.vector.memset(big[:, dt, 0:2], 0.0)
                                nc.scalar.activation(big[:, dt, 0:2],
                                                     big[:, dt, 0:2],
                                                     AF.Identity,
                                                     bias=bT[:, dt:dt + 1])
                    if c == NCH - 1:
                        for big, bT in ((kc, bkT), (vc, bvT)):
                            for dt in range(DT):
                                nc.vector.memset(big[:, dt, CHUNK + 2:CHUNK + 4], 0.0)
                                nc.scalar.activation(big[:, dt, CHUNK + 2:CHUNK + 4],
                                                     big[:, dt, CHUNK + 2:CHUNK + 4],
                                                     AF.Identity,
                                                     bias=bT[:, dt:dt + 1])
                    # zero the 2-col align pad so the shifted copies read defined data
                    nc.vector.memset(kc[:, :, CHUNK + 4:KW], 0.0)
                    nc.vector.memset(vc[:, :, CHUNK + 4:KW], 0.0)
                    return x_sb, qT

                def attention(c, x_sb, qT):
                    """scores/softmax/AV/O-proj/LN for chunk c (projections done)."""
                    s0 = c * CHUNK
                    kc, vc = kc_tiles[c], vc_tiles[c]
                    # odd-tap alignment shadows: kcB[j] = kc[j+1] (4x-mode copies)
                    kcB = kvbpool.tile([128, DT, KW - 2], bf16, tag="kcB")
                    vcB = kvbpool.tile([128, DT, KW - 2], bf16, tag="vcB")
                    nc.vector.tensor_copy(kcB[:], kc[:, :, 1:KW - 1])
                    nc.vector.tensor_copy(vcB[:], vc[:, :, 1:KW - 1])

                    def tap(big, bigB, w):
                        """[128, DT, CHUNK] slice for tap w, always 4B-aligned."""
                        if w % 2 == 0:
                            return big[:, :, w:w + CHUNK]
                        return bigB[:, :, w - 1:w - 1 + CHUNK]

                    # prods: dt-merged elementwise q*k per tap (bf16 2x mode)
                    prods = []
                    for w in range(W):
                        pr = prodpool.tile([128, DT, CHUNK], bf16, tag="prod")
                        nc.vector.tensor_tensor(pr[:], qT[:], tap(kc, kcB, w),
                                                ALU.mult)
                        prods.append(pr)
                    # scores + head-reduce + broadcast; 2-tap 1-bank PSUM
                    # tiles so score matmuls pipeline with the exp activations
                    ex_all = expool.tile([128, DT, W, CHUNK], bf16, tag="ex")
                    for dt in range(DT):
                        for w0 in (0, 2, 4):
                            nw = 2 if w0 < 4 else 1
                            sc = spsum.tile([128, 2, CHUNK], f32, tag="scores")
                            for w in range(w0, w0 + nw):
                                nc.tensor.matmul(sc[:, w - w0, :], blockones[:],
                                                 prods[w][:, dt, :])
                            nc.scalar.activation(
                                ex_all[:, dt, w0:w0 + nw, :],
                                sc[:, 0:nw, :], AF.Exp, scale=0.125)
                    # softmax denominator (dt-merged bf16 adds) + reciprocal
                    dn = dnpool.tile([128, DT, CHUNK], bf16, tag="dn")
                    dnf = dnpool.tile([128, DT, CHUNK], f32, tag="dnf")
                    rinv = dnpool.tile([128, DT, CHUNK], f32, tag="rinv")
                    rinvb = dnpool.tile([128, DT, CHUNK], bf16, tag="rinvb")
                    nc.vector.tensor_tensor(dn[:], ex_all[:, :, 0, :],
                                            ex_all[:, :, 1, :], ALU.add)
                    nc.vector.tensor_tensor(dn[:], dn[:], ex_all[:, :, 2, :],
                                            ALU.add)
                    nc.vector.tensor_tensor(dn[:], dn[:], ex_all[:, :, 3, :],
                                            ALU.add)
                    nc.vector.tensor_tensor(dnf[:], dn[:], ex_all[:, :, 4, :],
                                            ALU.add)
                    nc.vector.reciprocal_approx_fast(rinv[:], dnf[:])
                    nc.vector.tensor_copy(rinvb[:], rinv[:])
                    # AV: avp_w = exp_w * v_tap_w (dt-merged), tree-sum, normalize
                    att = atpool.tile([128, DT, CHUNK], bf16, tag="att")
                    asum = avpool.tile([128, DT, CHUNK], bf16, tag="asum")
                    avp = avpool.tile([128, DT, CHUNK], bf16, tag="avp")
                    nc.vector.tensor_tensor(asum[:], ex_all[:, :, 0, :],
                                            tap(vc, vcB, 0), ALU.mult)
                    for w in range(1, W):
                        nc.vector.tensor_tensor(avp[:], ex_all[:, :, w, :],
                                                tap(vc, vcB, w), ALU.mult)
                        nc.vector.tensor_tensor(asum[:], asum[:], avp[:], ALU.add)
                    nc.vector.tensor_tensor(att[:], asum[:], rinvb[:], ALU.mult)

                    # O-projection + bias + residual + LayerNorm per s-tile
                    for st in range(2):
                        op = opsum.tile([128, D], f32, tag="o")
                        for dt in range(DT):
                            a_blk = att[:, dt, st * 128:(st + 1) * 128]
                            nc.tensor.matmul(op[:, 0:512], a_blk,
                                             wo_sb[:, dt, 0:512],
                                             start=(dt == 0), stop=False)
                            nc.tensor.matmul(op[:, 512:D], a_blk,
                                             wo_sb[:, dt, 512:D],
                                             start=(dt == 0), stop=False)
                        nc.tensor.matmul(op[:, 0:512], ones_bf[:],
                                         bo_bf[:, 0:512], start=False, stop=True)
                        nc.tensor.matmul(op[:, 512:D], ones_bf[:],
                                         bo_bf[:, 512:D], start=False, stop=True)
                        # residual (DVE: GPSIMD has no PSUM port)
                        ypre = ypool.tile([128, D], f32, tag="ypre")
                        nc.vector.tensor_tensor(ypre[:], op[:], x_sb[:, st, :],
                                                ALU.add)
                        # LayerNorm stats via bn_stats/bn_aggr (DVE one pass)
                        bns = stpool.tile([128, 2, 6], f32, tag="bns")
                        agg = stpool.tile([128, 2], f32, tag="agg")
                        stats = stpool.tile([128, 6], f32, tag="stats")
                        sti = stpool.tile([128, 2], mybir.dt.int32, tag="sti")
                        nc.vector.bn_stats(bns[:, 0, :], ypre[:, 0:384])
                        nc.vector.bn_stats(bns[:, 1, :], ypre[:, 384:D])
                        nc.vector.bn_aggr(agg[:], bns[:])
                        # rstd = rsqrt(var+eps) via bit-trick seed + 2 Newton
                        # iterations, all on GPSIMD (keeps ScalarE on one
                        # activation table set -- no ACT_TABLE_LOAD thrash)
                        xe = stats[:, 0:1]
                        y = stats[:, 1:2]
                        t = stats[:, 2:3]
                        h = stats[:, 3:4]
                        negmu = stats[:, 4:5]
                        nmr = stats[:, 5:6]
                        nc.gpsimd.tensor_tensor(xe, agg[:, 1:2], eps_sb[:],
                                                ALU.add)
                        nc.vector.tensor_scalar(sti[:, 0:1], xe.bitcast(
                            mybir.dt.int32), 1, None, ALU.arith_shift_right)
                        nc.vector.tensor_scalar(sti[:, 1:2], sti[:, 0:1], -1,
                                                0x5f3759df, ALU.mult, ALU.add)
                        nc.vector.tensor_scalar_mul(h, xe, -0.5)
                        nc.vector.tensor_scalar_mul(negmu, agg[:, 0:1], -1.0)
                        yseed = sti[:, 1:2].bitcast(f32)
                        # one NR iter: y = y0*(1.5 + h*y0*y0), h = -0.5*(var+eps)
                        # (seed err ~3.4% -> ~1.7e-3 after NR; fine vs 2e-2 gate)
                        nc.gpsimd.tensor_tensor(t, yseed, yseed, ALU.mult)
                        nc.gpsimd.tensor_tensor(t, t, h, ALU.mult)
                        nc.gpsimd.tensor_tensor(t, t, c15_sb[:], ALU.add)
                        nc.gpsimd.tensor_tensor(y, yseed, t, ALU.mult)
                        # negmurstd = -mean * rstd
                        nc.gpsimd.tensor_tensor(nmr, negmu, y, ALU.mult)
                        # y1 = ypre*rstd + (-mean*rstd): one DVE tensor_scalar
                        # with per-partition scale+bias (keeps ScalarE free)
                        y1 = ypool.tile([128, D], f32, tag="y1")
                        nc.vector.tensor_scalar(y1[:], ypre[:], y, nmr,
                                                ALU.mult, ALU.add)
                        y2 = ypool.tile([128, D], f32, tag="y2")
                        nc.gpsimd.tensor_tensor(y2[:], y1[:], gb_bc[:], ALU.mult)
                        nc.gpsimd.tensor_tensor(y2[:], y2[:], be_bc[:], ALU.add)
                        nc.sync.dma_start(
                            out_ap[s0 + st * 128: s0 + (st + 1) * 128, :], y2[:])

                # projections run TWO chunks ahead of attention so attention(c)
                # never waits on fresh projections (kcB needs the right halo
                # written by project(c+1)).  attention is emitted BEFORE the
                # next project so its (ready) DVE ops are not queued behind
                # project's halo copies, which wait on the new K-projection
                # (engine queues are strict FIFO -> head-of-line blocking).
                pend = []
                pend.append(project(0))
                pend.append(project(1))
                for c in range(2, NCH):
                    attention(c - 2, *pend[c - 2])
                    pend.append(project(c))
                attention(NCH - 2, *pend[NCH - 2])
                attention(NCH - 1, *pend[NCH - 1])

    nc.compile()
    return nc


def kernel(**inputs):
    if "nc" not in _cache:
        _cache["nc"] = _build()
    nc = _cache["nc"]
    from concourse.bass_utils import run_bass_kernel_spmd

    names = ["Wq", "bq", "Wk", "bk", "Wv", "bv", "Wo", "bo", "gamma", "beta"]
    shared = {n: np.ascontiguousarray(np.asarray(inputs[n], dtype=np.float32))
              for n in names}
    x = np.asarray(inputs["x"], dtype=np.float32)
    in_maps = [dict(shared, x=np.ascontiguousarray(x[b])) for b in range(N_CORES)]
    res = run_bass_kernel_spmd(nc, in_maps, core_ids=list(range(N_CORES)))
    out = np.stack([res.results[i]["out"] for i in range(N_CORES)], axis=0)
    return out.astype(np.float32)
